# revision 40
# baseline (speedup 1.0000x reference)
"""MetricSelfAttention TRN2 kernel, v2 (bf16 data plane).

Reference computation (b=2, w=2048, c=1024, n=16 heads, k=64):
    P        = softmax(tril_mask(pre_metric) / sqrt(k))      per head [k,k]
    metric   = P @ P^T
    proj     = x @ W_proj^T                                  (Q = K = V)
    scores   = (proj_h @ metric_h @ proj_h^T) / sqrt(k)      causal-masked softmax
    out      = (att @ proj) @ W_mix^T

scores = G @ G^T with G = proj_h @ P_h; transposed attention blocks
ET[j, i] = exp(s[j, i]/sqrt(k)) * causal(j <= i) are built directly in the
G^T layout, a ones-column appended to proj yields softmax row-sums from the
same matmul that accumulates att^T @ proj, and normalization is folded in
before the final mix matmul.

v2 changes vs v1:
  - bf16 everywhere off the PSUM accumulators: halves DMA traffic, removes
    the narrow-tile fp32r matmul penalty, and doubles several DVE ops.
  - G^T is computed straight from x^T with pre-multiplied weights
    Wg = W_proj_h^T @ P_h (tiny PE transform), eliminating the projT
    recompute pass and its PSUM->SBUF copies.
  - software-pipelined attention loop: scores(jp+1) is emitted before
    att^T@proj(jp), and filler matmuls (next i-block's proj/GT chains,
    previous i-block's mix) are drained one per jp so the PE rides out the
    per-block Exp overhead on the Activation engine.
  - causal mask applied by gpsimd affine_select (Pool engine), PSUM->SBUF
    copies split across DVE and Pool, reciprocal reads row-sums directly
    from PSUM, input DMA batched and spread across idle engine queues.

Sharding over 8 cores: core = (batch, head-group of 4).  Each core computes
a [2048, 1024] bf16 partial of the mix output for its 256 channels; the
host sums the 4 partials per batch in fp32.
"""

import numpy as np
import ml_dtypes

import concourse.bass as bass
import concourse.mybir as mybir
import concourse.tile as tile
from concourse.bass_utils import run_bass_kernel_spmd

B, W, C, NH, K = 2, 2048, 1024, 16, 64
HPC = 4            # heads per core
CPC = HPC * K      # 256 channels per core
F32 = mybir.dt.float32
F32R = mybir.dt.float32r
I32 = mybir.dt.int32
I16 = mybir.dt.int16
BF16 = mybir.dt.bfloat16
SCALE = 1.0 / 8.0  # 1/sqrt(K)
EXPA = SCALE * 1.4426950408889634 * 128  # bit-trick exp slope


def _split_waits(nc, max_waits=1):
    """Hoist extra sem waits onto wait-only EventSemaphore carriers.

    The walrus build here rejects any instruction carrying more than one
    sync wait ("Too many sync wait commands"), while Tile's add_semaphores
    freely packs several waits onto one instruction.  An EVSEM executes on
    the engine's sequencer, so program order still gates the instruction
    that originally carried the waits.
    """
    n_new = 0
    for f in nc.m.functions:
        for b in f.blocks:
            out = []
            changed = False
            for inst in b.instructions:
                si = inst.sync_info
                if si is not None and si.on_wait and len(si.on_wait) > max_waits:
                    waits = list(si.on_wait)
                    for w in waits[:-max_waits]:
                        n_new += 1
                        ev = mybir.InstEventSemaphore(
                            name=f"splitw_{n_new}_{inst.name}",
                            engine=inst.engine,
                            ins=[], outs=[],
                            sync_info=mybir.SyncInfo(on_wait=[w], on_update=[]),
                        )
                        out.append(ev)
                        changed = True
                    si.on_wait = waits[-max_waits:]
                out.append(inst)
            if changed:
                b.instructions = out
    return n_new


def build_nc(split_waits=True):
    # input layouts are pre-packed on the host to match the SBUF tiles
    # exactly, so every DMA line is >=1KB contiguous (512B lines ran the
    # fabric at ~half rate and tripled the descriptor count)
    nc = bass.Bass()
    xT_d = nc.dram_tensor("xTp", [2, 8, 128, 4, 256], BF16,
                          kind="ExternalInput")
    wpT_d = nc.dram_tensor("wpT", [128, 8, CPC], BF16, kind="ExternalInput")
    pmat_d = nc.dram_tensor("pmat", [128, 2, 128], BF16, kind="ExternalInput")
    wg_d = nc.dram_tensor("wg", [128, 8, CPC], BF16, kind="ExternalInput")
    wmT_d = nc.dram_tensor("wmT", [128, 2, C], BF16, kind="ExternalInput")
    out_d = nc.dram_tensor("partial", [W, C], BF16, kind="ExternalOutput")

    ge = mybir.AluOpType.is_ge
    Exp = mybir.ActivationFunctionType.Exp

    with tile.TileContext(nc) as tc:
        with (
            tc.tile_pool(name="big", bufs=1) as big,
            tc.tile_pool(name="work", bufs=2) as work,
            tc.tile_pool(name="et", bufs=6) as etp,
            tc.tile_pool(name="pp", bufs=1, space="PSUM") as pp,
        ):
            # ---- input loads: arrival priority is wpT + Wg + the first
            # 512 cols of x^T (proj and GT wf0), then the rest of x^T in
            # 256-col pieces, W_mix^T last (first needed ~25us in).  Issuance
            # is spread over the SP and Activation queues; everything x^T
            # moves in 256-col pieces, which the DMA fabric services at
            # twice the byte-rate of 512-col slices.
            wpT = big.tile([128, 8, CPC], BF16, name="wpT")
            nc.sync.dma_start(wpT[:, 0:2], wpT_d[:, 0:2])
            xTh = [big.tile([128, 4, W], BF16, name=f"xT{h}", tag=f"xT{h}")
                   for h in range(2)]
            xT = [xTh[ct // 4][:, ct % 4] for ct in range(8)]
            P_t = big.tile([128, 2, 128], BF16, name="P_t")
            Wg = big.tile([128, 8, CPC], BF16, name="Wg")
            wmT = big.tile([128, 2, C], BF16, name="wmT")
            nc.scalar.dma_start(xTh[0][:, :, 0:256], xT_d[0, 0])
            nc.sync.dma_start(xTh[1][:, :, 0:256], xT_d[1, 0])
            nc.sync.dma_start(wpT[:, 2:8], wpT_d[:, 2:8])
            nc.scalar.dma_start(xTh[0][:, :, 256:512], xT_d[0, 1])
            nc.sync.dma_start(xTh[1][:, :, 256:512], xT_d[1, 1])
            nc.sync.dma_start(Wg, wg_d[:, :, :])
            for q in range(2, 8):
                nc.scalar.dma_start(xTh[0][:, :, 256 * q:256 * (q + 1)],
                                    xT_d[0, q])
                nc.sync.dma_start(xTh[1][:, :, 256 * q:256 * (q + 1)],
                                  xT_d[1, q])
            nc.sync.dma_start(P_t, pmat_d[:, :, :])
            nc.sync.dma_start(wmT, wmT_d[:, :, :])

            # a throwaway Exp warms the activation table while DMAs stream
            warm = big.tile([1, 2], F32, name="warm")
            nc.vector.memset(warm, 1.0)
            nc.scalar.activation(warm, warm, Exp, scale=1.0)
            # recip-broadcast setup: recip rows live on partitions 0 and 32
            # of a plane per (i-block, head-pair), and one K=33 selector
            # matmul broadcasts both onto their 64-partition blocks.  Both
            # heads' rowsums are staged into one [33,512] tile (partitions
            # 0/32; lanes 1..31 hold 1.0 from a one-time fill) so a single
            # 33-lane reciprocal covers both heads and the rbc matmul's dead
            # lanes see finite recip(1.0), not NaN-tainted garbage.
            sel33 = big.tile([64, 128], F32R, name="sel33")
            nc.vector.memset(sel33.bitcast(F32), 0.0)
            nc.vector.memset(sel33.bitcast(F32)[0:1, 0:K], 1.0)
            nc.vector.memset(sel33.bitcast(F32)[32:33, K:128], 1.0)
            rw_all = big.tile([64, 8, 512], F32R, name="rw_all")
            rsum2 = big.tile([33, 2, 512], F32, name="rsum2")
            nc.gpsimd.memset(rsum2[0:32, :], 1.0)

            # ---- proj row-tiles and GT = (x @ Wg)^T -------------------------
            # proj tile wt: [128 w-rows, (h, k)] + ones column for row-sums.
            # Next i-block's proj/GT chains and the previous i-block's mix
            # tiles flow through a FIFO filler queue drained one matmul per
            # jp of the attention loop (PSUM via the then-idle "mix" tag).
            proj = [None] * 16
            tr_by_wt = {}
            projT_all = big.tile([128, 2, W], BF16, name="projT")
            GT = [big.tile([128, W], BF16, name=f"GT{g}", tag=f"GT{g}")
                  for g in range(2)]
            gt_done = set()
            filler = []          # items: (opens_chain, fn)

            def emit_proj(wt, tag):
                box = {}

                def mm(ct):
                    if ct == 0:
                        ps = pp.tile([128, 2, 512], F32, tag=tag,
                                     name="ps_proj", bufs=2) \
                            if tag == "sc" else \
                            pp.tile([128, 512], F32, tag=tag,
                                    name="ps_projf", bufs=2)
                        box['ps'] = ps[:, 0] if tag == "sc" else ps
                    nc.tensor.matmul(
                        box['ps'][:, 0:CPC],
                        lhsT=xT[ct][:, wt * 128:(wt + 1) * 128],
                        rhs=wpT[:, ct],
                        start=(ct == 0), stop=(ct == 7),
                    )
                    if ct == 7:
                        pt = big.tile([128, HPC, K + 1], BF16,
                                      name=f"proj{wt}", tag=f"proj{wt}")
                        nc.vector.tensor_copy(
                            out=pt[:, :, 0:K],
                            in_=box['ps'][:, 0:CPC].rearrange(
                                "p (h k) -> p h k", k=K),
                        )
                        nc.vector.memset(pt[:, :, K:K + 1], 1.0)
                        proj[wt] = pt
                        if wt < 4:
                            return  # wf0's GT comes from the direct path
                        # contiguous value copy feeds the xbar transpose
                        # that builds projT[k', h, w] for the GT matmuls.
                        # InstDmaTransposeAnt is invisible to the tile dep
                        # tracker, so its edges are added explicitly.
                        pv = work.tile([128, CPC], BF16, name="pv",
                                       tag="pv", bufs=3)
                        pvc = nc.vector.tensor_copy(out=pv,
                                                    in_=box['ps'][:, 0:CPC])
                        tr = nc.sync.dma_start_transpose(
                            projT_all[:, :, wt * 128:(wt + 1) * 128],
                            pv[:, :])
                        bass._add_dep_helper(
                            tr.ins, pvc.ins, sync=True,
                            reason="xbar transpose reads pv")
                        if wt - 3 in tr_by_wt:
                            bass._add_dep_helper(
                                pvc.ins, tr_by_wt[wt - 3].ins, sync=True,
                                reason="pv slot recycle")
                        tr_by_wt[wt] = tr
                return [(ct == 0, lambda ct=ct: mm(ct)) for ct in range(8)]

            def emit_gt(g, wf, tag):
                # wf=0 (startup): direct Wg^T @ x^T contraction -- projT
                # isn't ready until the input fabric drains.  wf>=1: two
                # tiny K=64 P^T @ projT matmuls per head-pair instead of
                # the full x^T re-contraction.
                if wf == 0:
                    box = {}

                    def mm0(ct):
                        if ct == 0:
                            ps = pp.tile([128, 2, 512], F32, tag=tag,
                                         name="ps_gt", bufs=2)
                            box['ps'] = ps[:, 0]
                        nc.tensor.matmul(
                            box['ps'],
                            lhsT=Wg[:, ct, 128 * g:128 * (g + 1)],
                            rhs=xT[ct][:, 0:512],
                            start=(ct == 0), stop=(ct == 7),
                        )
                        if ct == 7:
                            nc.vector.tensor_copy(
                                out=GT[g][:, 0:512], in_=box['ps'])
                            gt_done.add((g, 0))
                    return [(ct == 0, lambda ct=ct: mm0(ct))
                            for ct in range(8)]

                def unit():
                    if tag == "sc":
                        ps = pp.tile([128, 2, 512], F32, tag="sc",
                                     name="ps_gt", bufs=2)[:, 0]
                    else:
                        ps = pp.tile([128, 512], F32, tag=tag,
                                     name="ps_gtf", bufs=2)
                    mm = nc.tensor.matmul(
                        ps,
                        lhsT=P_t[:, g],
                        rhs=projT_all[:, g, wf * 512:(wf + 1) * 512],
                        start=True, stop=True,
                    )
                    bass._add_dep_helper(
                        mm.ins, tr_by_wt[4 * wf + 3].ins, sync=True,
                        reason="projT chunk ready (same-queue FIFO)")
                    nc.vector.tensor_copy(
                        out=GT[g][:, wf * 512:(wf + 1) * 512], in_=ps)
                    gt_done.add((g, wf))
                return [(True, unit)]

            def emit_mix(wt, dma_eng):
                box = {}

                def mmpair(mf):
                    if mf == 0:
                        box['t'] = work.tile([128, C], BF16, name="ost",
                                             tag="ost", bufs=3)
                    # the tail block also rotates through the then-idle
                    # "nud" banks so drains never block the matmul pairs
                    tag = "nud" if (dma_eng is None and mf == 1) else "mix"
                    ps = pp.tile([128, 512], F32, tag=tag, name="ps_mix",
                                 bufs=2)
                    for c2 in range(2):
                        nc.tensor.matmul(
                            ps,
                            lhsT=nudT[c2][:, wt * 128:(wt + 1) * 128],
                            rhs=wmT[:, c2, mf * 512:(mf + 1) * 512],
                            start=(c2 == 0), stop=(c2 == 1),
                        )
                    # PSUM drain: DVE, except the tail i-block where the
                    # Activation engine is out of exp work and takes half;
                    # tail halves DMA out independently on two queues so the
                    # final transfer is short and issued early
                    if dma_eng is None:
                        # tail: ship each half as soon as it's cast, on its
                        # own queue, so the final drain overlaps compute
                        nc.vector.tensor_copy(
                            out=box['t'][:, mf * 512:(mf + 1) * 512], in_=ps)
                        (nc.sync if mf == 0 else nc.scalar).dma_start(
                            out_d[wt * 128:(wt + 1) * 128,
                                  mf * 512:(mf + 1) * 512],
                            box['t'][:, mf * 512:(mf + 1) * 512])
                    else:
                        nc.vector.tensor_copy(
                            out=box['t'][:, mf * 512:(mf + 1) * 512], in_=ps)
                        if mf == 1:
                            dma_eng.dma_start(
                                out_d[wt * 128:(wt + 1) * 128, :], box['t'])
                # per-c2 granularity: each (mf, c2) is one filler unit so
                # pops interleave at 213ns steps; c2=0 opens the chain
                def half(mf, c2):
                    if mf == 0 and c2 == 0:
                        box['t'] = work.tile([128, C], BF16, name="ost",
                                             tag="ost", bufs=3)
                    if c2 == 0:
                        tag = "nud" if (dma_eng is None and mf == 1) else "mix"
                        box[mf] = pp.tile([128, 512], F32, tag=tag,
                                          name="ps_mix", bufs=2)
                    nc.tensor.matmul(
                        box[mf],
                        lhsT=nudT[c2][:, wt * 128:(wt + 1) * 128],
                        rhs=wmT[:, c2, mf * 512:(mf + 1) * 512],
                        start=(c2 == 0), stop=(c2 == 1),
                    )
                    if c2 == 1:
                        nc.vector.tensor_copy(
                            out=box['t'][:, mf * 512:(mf + 1) * 512],
                            in_=box[mf])
                        if mf == 1:
                            (nc.sync if dma_eng is None else
                             dma_eng).dma_start(
                                out_d[wt * 128:(wt + 1) * 128, :], box['t'])
                if dma_eng is None:
                    return [(True, lambda mf=mf: mmpair(mf))
                            for mf in range(2)]
                return [(c2 == 0, lambda mf=mf, c2=c2: half(mf, c2))
                        for mf in range(2) for c2 in range(2)]

            def pop_filler():
                filler.pop(0)[1]()

            def flush_until(pred):
                while filler and not pred():
                    pop_filler()

            def flush_chain_boundary():
                # emit until the queue head would OPEN a new psum chain, so
                # no half-accumulated "mix"-tag tile is left holding a slot
                # that the rbc/mix section below would deadlock on.
                while filler and not filler[0][0]:
                    pop_filler()

            def pop_n_to_boundary(n):
                for _ in range(n):
                    if not filler:
                        break
                    pop_filler()
                flush_chain_boundary()

            for wt in range(4):
                for _, f in emit_proj(wt, tag="sc"):
                    f()
            for g in range(2):
                for _, f in emit_gt(g, 0, tag="sc"):
                    f()

            # ---- main loop: attention -> normalize -> mix, per i-block -----
            # ET[j, i] = exp(scores[j, i]/8) * (j <= i), accumulated into
            # nudged^T via att^T @ proj; the ones-column gives row sums.
            nudT = [big.tile([128, W], BF16, name=f"nudT{g}", tag=f"nudT{g}")
                    for g in range(2)]

            def emit_norm(i2, g, rbc_tag="mix"):
                # one K=33 selector matmul broadcasts both heads' recip
                # rows onto their 64-partition blocks, then one multiply
                # normalizes the whole 128-partition i-block
                i2s = slice(i2 * 512, (i2 + 1) * 512)
                if rbc_tag == "sc":
                    rbc = pp.tile([128, 2, 512], F32, tag="sc",
                                  name="rbc_ps", bufs=2)[:, 0]
                else:
                    rbc = pp.tile([128, 512], F32, tag=rbc_tag,
                                  name="rbc_ps", bufs=2)
                nc.tensor.matmul(
                    rbc,
                    lhsT=sel33[0:33],
                    rhs=rw_all[0:33, 2 * i2 + g],
                    start=True, stop=True,
                )
                nc.vector.tensor_tensor(
                    nudT[g][:, i2s],
                    nudT[g][:, i2s],
                    rbc,
                    mybir.AluOpType.mult,
                )

            def emit_norm_mix(i2, tail=False):
                # normalize + mix for i-block i2, deferred into the middle of
                # the next i-block's attention so the recip/drain chain on
                # DVE/Act overlaps a whole jp loop instead of stalling the PE
                flush_chain_boundary()
                if tail:
                    # head-pair 0 was normalized before the last jp loop, so
                    # the first three mix tiles' c2=0 accumulations are legal
                    # PE work while the final recips run on DVE.  6 opened
                    # banks (mix 2 + nud 2 + one 2-bank "sc" alloc) plus the
                    # final rbc on the other "sc" alloc fill all of PSUM.
                    opened = []
                    for wt in (4 * i2, 4 * i2 + 1, 4 * i2 + 2):
                        ost = work.tile([128, C], BF16, name="ost",
                                        tag="ost", bufs=3)
                        sc_ps = (pp.tile([128, 2, 512], F32, tag="sc",
                                         name="ps_mixo_sc", bufs=2)
                                 if wt == 4 * i2 + 2 else None)
                        for mf in range(2):
                            if sc_ps is not None:
                                ps = sc_ps[:, mf]
                            else:
                                ps = pp.tile([128, 512], F32,
                                             tag="mix" if mf == 0 else "nud",
                                             name="ps_mixo", bufs=2)
                            nc.tensor.matmul(
                                ps,
                                lhsT=nudT[0][:, wt * 128:(wt + 1) * 128],
                                rhs=wmT[:, 0, mf * 512:(mf + 1) * 512],
                                start=True, stop=False,
                            )
                            opened.append((wt, mf, ps, ost))
                    emit_norm(i2, 1, rbc_tag="sc")
                    dmaq = [nc.sync, nc.scalar]
                    for qi, (wt, mf, ps, ost) in enumerate(opened):
                        nc.tensor.matmul(
                            ps,
                            lhsT=nudT[1][:, wt * 128:(wt + 1) * 128],
                            rhs=wmT[:, 1, mf * 512:(mf + 1) * 512],
                            start=False, stop=True,
                        )
                        eng = (nc.vector.tensor_copy if mf == 0 else
                               lambda out, in_: nc.scalar.activation(
                                   out, in_,
                                   mybir.ActivationFunctionType.Copy,
                                   scale=1.0))
                        eng(out=ost[:, mf * 512:(mf + 1) * 512], in_=ps)
                        dmaq[qi % 2].dma_start(
                            out_d[wt * 128:(wt + 1) * 128,
                                  mf * 512:(mf + 1) * 512],
                            ost[:, mf * 512:(mf + 1) * 512])
                    for _, f in emit_mix(4 * i2 + 3, None):
                        f()
                    return
                for g in range(2):
                    emit_norm(i2, g)
                # mix: half the row-tiles ride the filler queue, the rest are
                # emitted directly (the tail block drains everything now)
                for wt in range(4 * i2, 4 * i2 + 4):
                    filler.extend(emit_mix(wt, nc.sync))

            def emit_scores_for(iFx, gx, jp):
                # columns < 128*d are fully causal-masked for this
                # j-block: never computed, never accumulated.
                d = jp - 4 * iFx
                lo = 128 * d if d > 0 else 0
                sc = pp.tile([128, 2, 512], F32, tag="sc",
                             name="sc_ps", bufs=2)
                for s in range(2):
                    nc.tensor.matmul(
                        sc[:, s, lo:],
                        lhsT=GT[gx][64 * s:64 * s + 64,
                                    jp * 128:(jp + 1) * 128],
                        rhs=GT[gx][64 * s:64 * s + 64,
                                   iFx * 512 + lo:(iFx + 1) * 512],
                        start=True, stop=True,
                    )
                et = etp.tile([128, 2, 512], BF16, name="et", tag="et")
                nc.scalar.activation(et[:, :, lo:], sc[:, :, lo:],
                                     Exp, scale=SCALE)
                if d >= 0:
                    # zero the strictly-upper part of the 128-wide
                    # diagonal crossing strip (cols [lo, lo+128))
                    nc.gpsimd.affine_select(
                        out=et[:, :, lo:lo + 128],
                        in_=et[:, :, lo:lo + 128],
                        compare_op=ge, fill=0.0,
                        base=0, channel_multiplier=-1,
                        pattern=[[0, 2], [1, 128]],
                    )
                return et

            # pre-emitted score blocks for the NEXT (iF, g) section: the
            # first two j-blocks' scores+exp are issued during the previous
            # section's drain/recip window so Act's exp pipeline never
            # drains across the boundary and the PE re-enters a full
            # 2-deep pipeline immediately.
            pre_ets = None
            for iF in range(4):
                njp = 4 * iF + 4
                flush_until(lambda: (0, iF) in gt_done and (1, iF) in gt_done)
                if iF + 1 < 4:
                    for wt in range(4 * iF + 4, 4 * iF + 8):
                        for _, f in emit_proj(wt, tag="mix"):
                            f()
                    for g in range(2):
                        filler.extend(emit_gt(g, iF + 1, tag="mix"))
                for g in range(2):
                    npair = [
                        pp.tile([128, 512], F32, tag="nud", name=f"nud_ps{s}",
                                bufs=2)
                        for s in range(2)
                    ]

                    def emit_npair(jp, et):
                        d = jp - 4 * iF
                        lo = 128 * d if d > 0 else 0
                        for s in range(2):
                            nc.tensor.matmul(
                                npair[s][:K + 1, lo:],
                                lhsT=proj[jp][:, 2 * g + s],
                                rhs=et[:, s, lo:],
                                start=(jp == 0), stop=(jp == njp - 1),
                            )

                    # two-deep software pipeline: scores(jp+2) is emitted
                    # ahead of npair(jp) so the Exp and the diagonal mask
                    # never gate the PE; one filler matmul per jp rides out
                    # the Exp per-block overhead.
                    if pre_ets is not None:
                        ets = pre_ets
                        pre_ets = None
                    else:
                        ets = [emit_scores_for(iF, g, 0),
                               emit_scores_for(iF, g, 1)]
                    if g == 0 and iF > 0:
                        # the previous i-block's npair PSUM frees only after
                        # its recip/drain chain; filler rides that out
                        pop_n_to_boundary(4)
                    for jp in range(njp):
                        if jp + 2 < njp:
                            ets.append(emit_scores_for(iF, g, jp + 2))
                        flush_until(lambda: proj[jp] is not None)
                        emit_npair(jp, ets[jp])
                        # early i-blocks have few jps but a full complement
                        # of next-block proj/GT fillers: drain two per jp
                        # there so the projT transposes start early enough
                        # to not gate the next i-block's GT
                        if filler and (iF < 3 or len(filler) > 6):
                            pop_filler()
                        if iF < 2 and filler:
                            pop_filler()

                    # stage both rowsum rows (parallel DVE+Act copies) so
                    # the npair PSUM banks free in ~0.5us.  The reciprocal
                    # is 4 short DVE ops (bitwise-NOT seed + one tuned
                    # Newton step, max rel err ~2e-3 -- invisible under the
                    # bf16 data plane) instead of the microcoded RECIPROCAL
                    # whose ~3.4us runtime stalled whatever queued behind it.
                    pl = (2 * iF + g) % 2
                    nc.vector.tensor_copy(out=rsum2[0:1, pl],
                                          in_=npair[0][K:K + 1])
                    nc.scalar.activation(
                        rsum2[32:33, pl], npair[1][K:K + 1],
                        mybir.ActivationFunctionType.Copy, scale=1.0)

                    def emit_recip(iF=iF, g=g, pl=pl):
                        C0, C1 = -0.23549792, 2.0017324
                        x = rsum2[0:33, pl]
                        t0 = work.tile([33, 512], F32, name="rt0",
                                       tag="rt0", bufs=2)
                        w1 = work.tile([33, 512], F32, name="rw1",
                                       tag="rw1", bufs=2)
                        nc.vector.tensor_scalar(
                            out=t0.bitcast(I32), in0=x.bitcast(I32),
                            scalar1=0, scalar2=None,
                            op0=mybir.AluOpType.bitwise_not)
                        nc.vector.tensor_tensor(
                            w1, x, t0, mybir.AluOpType.mult)
                        nc.vector.tensor_scalar(
                            out=w1, in0=w1, scalar1=C0, scalar2=C1,
                            op0=mybir.AluOpType.mult,
                            op1=mybir.AluOpType.subtract)
                        with nc.allow_low_precision(
                                reason="f32r recip feeds f32r matmul"):
                            nc.vector.scalar_tensor_tensor(
                                out=rw_all[0:33, 2 * iF + g],
                                in0=w1, scalar=-C0, in1=t0,
                                op0=mybir.AluOpType.mult,
                                op1=mybir.AluOpType.mult)

                    for s in range(2):
                        # g=1 drains land in the i-block boundary window
                        # where Act has no exp backlog but DVE is piled up
                        dst = nudT[g][64 * s:64 * s + 64,
                                      iF * 512:(iF + 1) * 512]
                        if g == 1:
                            nc.scalar.activation(
                                dst, npair[s][:K],
                                mybir.ActivationFunctionType.Copy, scale=1.0)
                        else:
                            nc.vector.tensor_copy(out=dst, in_=npair[s][:K])

                    emit_recip()
                    if g == 0:
                        pre_ets = [emit_scores_for(iF, 1, 0),
                                   emit_scores_for(iF, 1, 1)]
                        if iF > 0:
                            emit_norm_mix(iF - 1)
                        if iF == 3:
                            # normalize the last block's first head-pair
                            # before the second pair's jp loop, off the
                            # tail chain
                            flush_chain_boundary()
                            emit_norm(3, 0)


            flush_until(lambda: False)
            emit_norm_mix(3, tail=True)
    if split_waits:
        _split_waits(nc)
    return nc


_NC_CACHE = None


def _get_nc():
    global _NC_CACHE
    if _NC_CACHE is None:
        _NC_CACHE = build_nc()
    return _NC_CACHE


def make_in_maps(in_sequence_bwc, W_proj, pre_metric, W_mix):
    bf = ml_dtypes.bfloat16
    # weight-only preprocessing: P_h = softmax(tril(pre_metric_h)/sqrt(k));
    # G^T = P^T @ proj^T is built on-device from xbar-transposed proj.
    pmf = np.asarray(pre_metric, np.float64)
    pmf = np.where(np.tril(np.ones((K, K), bool)), pmf, -np.inf) / np.sqrt(K)
    pmf = pmf - pmf.max(-1, keepdims=True)
    P = np.exp(pmf)
    P /= P.sum(-1, keepdims=True)                       # [NH, K, K]
    WgT = np.einsum('nkc,nkl->nlc', W_proj.reshape(NH, K, C).astype(np.float64),
                    P)                                  # [NH, K(l), C]
    in_maps = []
    for core in range(8):
        b, hg = core // 4, core % 4
        cs = slice(CPC * hg, CPC * (hg + 1))
        pmat = np.zeros((128, 2, 128), np.float64)      # blockdiag P pairs
        for hl in range(4):
            g_, s_ = hl // 2, hl % 2
            pmat[64 * s_:64 * s_ + 64, g_,
                 64 * s_:64 * s_ + 64] = P[4 * hg + hl]
        wg = WgT[4 * hg:4 * hg + 4].reshape(CPC, C).T   # [C, CPC]
        # pack to the kernel's SBUF layouts so DMA lines are contiguous:
        #   xTp[h, q, p, g, w'] = x^T[128*(4h+g)+p, 256q+w']
        #   wpT/wg[p, g, m]     = (.)[128g+p, m]
        #   wmT[ci, co, m]      = W_mix[:, cs].T[128co+ci, m]
        xbT = in_sequence_bwc[b].T.astype(bf)           # [C, W]
        xTp = xbT.reshape(2, 4, 128, 8, 256).transpose(0, 3, 2, 1, 4)
        wpTp = W_proj[cs, :].T.reshape(8, 128, CPC).transpose(1, 0, 2)
        wgp = wg.reshape(8, 128, CPC).transpose(1, 0, 2)
        wmTp = W_mix[:, cs].T.reshape(2, 128, C).transpose(1, 0, 2)
        in_maps.append({
            "xTp": np.ascontiguousarray(xTp),
            "wpT": np.ascontiguousarray(wpTp.astype(bf)),
            "pmat": np.ascontiguousarray(pmat.astype(bf)),
            "wg": np.ascontiguousarray(wgp.astype(bf)),
            "wmT": np.ascontiguousarray(wmTp.astype(bf)),
        })
    return in_maps


def combine_results(results):
    out = np.zeros((B, W, C), np.float32)
    for core in range(8):
        out[core // 4] += np.asarray(results[core]["partial"], np.float32)
    return out


def kernel(in_sequence_bwc, W_proj, pre_metric, W_mix):
    nc = _get_nc()
    in_maps = make_in_maps(
        np.asarray(in_sequence_bwc), np.asarray(W_proj),
        np.asarray(pre_metric), np.asarray(W_mix),
    )
    res = run_bass_kernel_spmd(nc, in_maps, list(range(8))).results
    return combine_results(res)



# revision 42
# speedup vs baseline: 1.0146x; 1.0146x over previous
"""MetricSelfAttention TRN2 kernel, v3 (bf16 data plane).

Reference computation (b=2, w=2048, c=1024, n=16 heads, k=64):
    P        = softmax(tril_mask(pre_metric) / sqrt(k))      per head [k,k]
    metric   = P @ P^T
    proj     = x @ W_proj^T                                  (Q = K = V)
    scores   = (proj_h @ metric_h @ proj_h^T) / sqrt(k)      causal-masked softmax
    out      = (att @ proj) @ W_mix^T

scores = G @ G^T with G = proj_h @ P_h; transposed attention blocks
ET[j, i] = exp(s[j, i]/sqrt(k)) * causal(j <= i) are built directly in the
G^T layout, a ones-column appended to proj yields softmax row-sums from the
same matmul that accumulates att^T @ proj, and normalization is folded in
before the final mix matmul.

v2 changes vs v1:
  - bf16 everywhere off the PSUM accumulators: halves DMA traffic, removes
    the narrow-tile fp32r matmul penalty, and doubles several DVE ops.
  - G^T is computed straight from x^T with pre-multiplied weights
    Wg = W_proj_h^T @ P_h (tiny PE transform), eliminating the projT
    recompute pass and its PSUM->SBUF copies.
  - software-pipelined attention loop: scores(jp+1) is emitted before
    att^T@proj(jp), and filler matmuls (next i-block's proj/GT chains,
    previous i-block's mix) are drained one per jp so the PE rides out the
    per-block Exp overhead on the Activation engine.
  - causal mask applied by gpsimd affine_select (Pool engine), PSUM->SBUF
    copies split across DVE and Pool, reciprocal reads row-sums directly
    from PSUM, input DMA batched and spread across idle engine queues.

v3 changes vs v2 (193us -> 148us):
  - softmax reciprocal rebuilt: both heads' row-sum rows are staged off
    PSUM in ~0.5us (parallel DVE+Act copies, freeing the npair banks the
    PE was stalling on), then ONE 33-lane bitwise-NOT-seeded Newton
    reciprocal (4 short DVE ops, ~2e-3 max err) replaces two 3.4us
    single-lane microcoded RECIPROCALs whose latency serialized the rbc
    matmuls, GT casts and PSUM recycling behind them (~30us of PE idle).
  - host-side input repack: every dram tensor is laid out exactly as its
    SBUF tile ([p][g][m] etc.), so DMA lines are 1-4KB contiguous instead
    of 512B -- less descriptor pressure, faster startup fill.
  - tail output is shipped per-[128,512] half on three queues as soon as
    each cast lands, instead of whole [128,1024] tiles on one queue.

Sharding over 8 cores: core = (batch, head-group of 4).  Each core computes
a [2048, 1024] bf16 partial of the mix output for its 256 channels; the
host sums the 4 partials per batch in fp32.
"""

import numpy as np
import ml_dtypes

import concourse.bass as bass
import concourse.mybir as mybir
import concourse.tile as tile
from concourse.bass_utils import run_bass_kernel_spmd

B, W, C, NH, K = 2, 2048, 1024, 16, 64
HPC = 4            # heads per core
CPC = HPC * K      # 256 channels per core
F32 = mybir.dt.float32
F32R = mybir.dt.float32r
I32 = mybir.dt.int32
BF16 = mybir.dt.bfloat16
SCALE = 1.0 / 8.0  # 1/sqrt(K)


def _split_waits(nc, max_waits=1):
    """Hoist extra sem waits onto wait-only EventSemaphore carriers.

    The walrus build here rejects any instruction carrying more than one
    sync wait ("Too many sync wait commands"), while Tile's add_semaphores
    freely packs several waits onto one instruction.  An EVSEM executes on
    the engine's sequencer, so program order still gates the instruction
    that originally carried the waits.
    """
    n_new = 0
    for f in nc.m.functions:
        for b in f.blocks:
            out = []
            changed = False
            for inst in b.instructions:
                si = inst.sync_info
                if si is not None and si.on_wait and len(si.on_wait) > max_waits:
                    waits = list(si.on_wait)
                    for w in waits[:-max_waits]:
                        n_new += 1
                        ev = mybir.InstEventSemaphore(
                            name=f"splitw_{n_new}_{inst.name}",
                            engine=inst.engine,
                            ins=[], outs=[],
                            sync_info=mybir.SyncInfo(on_wait=[w], on_update=[]),
                        )
                        out.append(ev)
                        changed = True
                    si.on_wait = waits[-max_waits:]
                out.append(inst)
            if changed:
                b.instructions = out
    return n_new


def build_nc(split_waits=True):
    # input layouts are pre-packed on the host to match the SBUF tiles
    # exactly, so every DMA line is >=1KB contiguous (512B lines ran the
    # fabric at ~half rate and tripled the descriptor count)
    nc = bass.Bass()
    xT_d = nc.dram_tensor("xTp", [2, 8, 128, 4, 256], BF16,
                          kind="ExternalInput")
    wpT_d = nc.dram_tensor("wpT", [128, 8, CPC], BF16, kind="ExternalInput")
    pmat_d = nc.dram_tensor("pmat", [128, 2, 128], BF16, kind="ExternalInput")
    wg_d = nc.dram_tensor("wg", [128, 8, CPC], BF16, kind="ExternalInput")
    wmT_d = nc.dram_tensor("wmT", [128, 2, C], BF16, kind="ExternalInput")
    out_d = nc.dram_tensor("partial", [W, C], BF16, kind="ExternalOutput")

    ge = mybir.AluOpType.is_ge
    Exp = mybir.ActivationFunctionType.Exp

    with tile.TileContext(nc) as tc:
        with (
            tc.tile_pool(name="big", bufs=1) as big,
            tc.tile_pool(name="work", bufs=2) as work,
            tc.tile_pool(name="et", bufs=6) as etp,
            tc.tile_pool(name="pp", bufs=1, space="PSUM") as pp,
        ):
            # ---- input loads: arrival priority is wpT + Wg + the first
            # 512 cols of x^T (proj and GT wf0), then the rest of x^T in
            # 256-col pieces, W_mix^T last (first needed ~25us in).  Issuance
            # is spread over the SP and Activation queues; everything x^T
            # moves in 256-col pieces, which the DMA fabric services at
            # twice the byte-rate of 512-col slices.
            wpT = big.tile([128, 8, CPC], BF16, name="wpT")
            nc.sync.dma_start(wpT[:, 0:2], wpT_d[:, 0:2])
            xTh = [big.tile([128, 4, W], BF16, name=f"xT{h}", tag=f"xT{h}")
                   for h in range(2)]
            xT = [xTh[ct // 4][:, ct % 4] for ct in range(8)]
            P_t = big.tile([128, 2, 128], BF16, name="P_t")
            Wg = big.tile([128, 8, CPC], BF16, name="Wg")
            wmT = big.tile([128, 2, C], BF16, name="wmT")
            nc.scalar.dma_start(xTh[0][:, :, 0:256], xT_d[0, 0])
            nc.sync.dma_start(xTh[1][:, :, 0:256], xT_d[1, 0])
            nc.sync.dma_start(wpT[:, 2:8], wpT_d[:, 2:8])
            nc.sync.dma_start(Wg, wg_d[:, :, :])
            nc.scalar.dma_start(xTh[0][:, :, 256:512], xT_d[0, 1])
            nc.sync.dma_start(xTh[1][:, :, 256:512], xT_d[1, 1])
            for q in range(2, 8):
                nc.scalar.dma_start(xTh[0][:, :, 256 * q:256 * (q + 1)],
                                    xT_d[0, q])
                nc.sync.dma_start(xTh[1][:, :, 256 * q:256 * (q + 1)],
                                  xT_d[1, q])
            nc.sync.dma_start(P_t, pmat_d[:, :, :])
            nc.gpsimd.dma_start(wmT, wmT_d[:, :, :])

            # a throwaway Exp warms the activation table while DMAs stream
            warm = big.tile([1, 2], F32, name="warm")
            nc.vector.memset(warm, 1.0)
            nc.scalar.activation(warm, warm, Exp, scale=1.0)
            # recip-broadcast setup: recip rows live on partitions 0 and 32
            # of a plane per (i-block, head-pair), and one K=33 selector
            # matmul broadcasts both onto their 64-partition blocks.  Both
            # heads' rowsums are staged into one [33,512] tile (partitions
            # 0/32; lanes 1..31 hold 1.0 from a one-time fill) so a single
            # 33-lane reciprocal covers both heads and the rbc matmul's dead
            # lanes see finite recip(1.0), not NaN-tainted garbage.
            sel33 = big.tile([64, 128], F32R, name="sel33")
            nc.vector.memset(sel33.bitcast(F32), 0.0)
            nc.vector.memset(sel33.bitcast(F32)[0:1, 0:K], 1.0)
            nc.vector.memset(sel33.bitcast(F32)[32:33, K:128], 1.0)
            rw_all = big.tile([64, 8, 512], F32R, name="rw_all")
            rsum2 = big.tile([33, 2, 512], F32, name="rsum2")
            nc.gpsimd.memset(rsum2[0:32, :], 1.0)

            # ---- proj row-tiles and GT = (x @ Wg)^T -------------------------
            # proj tile wt: [128 w-rows, (h, k)] + ones column for row-sums.
            # Next i-block's proj/GT chains and the previous i-block's mix
            # tiles flow through a FIFO filler queue drained one matmul per
            # jp of the attention loop (PSUM via the then-idle "mix" tag).
            proj = [None] * 16
            tr_by_wt = {}
            projT_all = big.tile([128, 2, W], BF16, name="projT")
            GT = [big.tile([128, W], BF16, name=f"GT{g}", tag=f"GT{g}")
                  for g in range(2)]
            gt_done = set()
            filler = []          # items: (opens_chain, fn)

            def emit_proj(wt, tag):
                box = {}

                def mm(ct):
                    if ct == 0:
                        ps = pp.tile([128, 2, 512], F32, tag=tag,
                                     name="ps_proj", bufs=2) \
                            if tag == "sc" else \
                            pp.tile([128, 512], F32, tag=tag,
                                    name="ps_projf", bufs=2)
                        box['ps'] = ps[:, 0] if tag == "sc" else ps
                    nc.tensor.matmul(
                        box['ps'][:, 0:CPC],
                        lhsT=xT[ct][:, wt * 128:(wt + 1) * 128],
                        rhs=wpT[:, ct],
                        start=(ct == 0), stop=(ct == 7),
                    )
                    if ct == 7:
                        pt = big.tile([128, HPC, K + 1], BF16,
                                      name=f"proj{wt}", tag=f"proj{wt}")
                        nc.vector.tensor_copy(
                            out=pt[:, :, 0:K],
                            in_=box['ps'][:, 0:CPC].rearrange(
                                "p (h k) -> p h k", k=K),
                        )
                        nc.vector.memset(pt[:, :, K:K + 1], 1.0)
                        proj[wt] = pt
                        if wt < 4:
                            return  # wf0's GT comes from the direct path
                        # contiguous value copy feeds the xbar transpose
                        # that builds projT[k', h, w] for the GT matmuls.
                        # InstDmaTransposeAnt is invisible to the tile dep
                        # tracker, so its edges are added explicitly.
                        pv = work.tile([128, CPC], BF16, name="pv",
                                       tag="pv", bufs=3)
                        pvc = nc.vector.tensor_copy(out=pv,
                                                    in_=box['ps'][:, 0:CPC])
                        tr = nc.sync.dma_start_transpose(
                            projT_all[:, :, wt * 128:(wt + 1) * 128],
                            pv[:, :])
                        bass._add_dep_helper(
                            tr.ins, pvc.ins, sync=True,
                            reason="xbar transpose reads pv")
                        if wt - 3 in tr_by_wt:
                            bass._add_dep_helper(
                                pvc.ins, tr_by_wt[wt - 3].ins, sync=True,
                                reason="pv slot recycle")
                        tr_by_wt[wt] = tr
                return [(ct == 0, lambda ct=ct: mm(ct)) for ct in range(8)]

            def emit_gt(g, wf, tag):
                # wf=0 (startup): direct Wg^T @ x^T contraction -- projT
                # isn't ready until the input fabric drains.  wf>=1: two
                # tiny K=64 P^T @ projT matmuls per head-pair instead of
                # the full x^T re-contraction.
                if wf == 0:
                    box = {}

                    def mm0(ct):
                        if ct == 0:
                            ps = pp.tile([128, 2, 512], F32, tag=tag,
                                         name="ps_gt", bufs=2)
                            box['ps'] = ps[:, 0]
                        nc.tensor.matmul(
                            box['ps'],
                            lhsT=Wg[:, ct, 128 * g:128 * (g + 1)],
                            rhs=xT[ct][:, 0:512],
                            start=(ct == 0), stop=(ct == 7),
                        )
                        if ct == 7:
                            nc.vector.tensor_copy(
                                out=GT[g][:, 0:512], in_=box['ps'])
                            gt_done.add((g, 0))
                    return [(ct == 0, lambda ct=ct: mm0(ct))
                            for ct in range(8)]

                def unit():
                    if tag == "sc":
                        ps = pp.tile([128, 2, 512], F32, tag="sc",
                                     name="ps_gt", bufs=2)[:, 0]
                    else:
                        ps = pp.tile([128, 512], F32, tag=tag,
                                     name="ps_gtf", bufs=2)
                    mm = nc.tensor.matmul(
                        ps,
                        lhsT=P_t[:, g],
                        rhs=projT_all[:, g, wf * 512:(wf + 1) * 512],
                        start=True, stop=True,
                    )
                    bass._add_dep_helper(
                        mm.ins, tr_by_wt[4 * wf + 3].ins, sync=True,
                        reason="projT chunk ready (same-queue FIFO)")
                    nc.vector.tensor_copy(
                        out=GT[g][:, wf * 512:(wf + 1) * 512], in_=ps)
                    gt_done.add((g, wf))
                return [(True, unit)]

            def emit_mix(wt, dma_eng):
                box = {}

                def mmpair(mf):
                    if mf == 0:
                        box['t'] = work.tile([128, C], BF16, name="ost",
                                             tag="ost", bufs=3)
                    # the tail block also rotates through the then-idle
                    # "nud" banks so drains never block the matmul pairs
                    tag = "nud" if (dma_eng is None and mf == 1) else "mix"
                    ps = pp.tile([128, 512], F32, tag=tag, name="ps_mix",
                                 bufs=2)
                    for c2 in range(2):
                        nc.tensor.matmul(
                            ps,
                            lhsT=nudT[c2][:, wt * 128:(wt + 1) * 128],
                            rhs=wmT[:, c2, mf * 512:(mf + 1) * 512],
                            start=(c2 == 0), stop=(c2 == 1),
                        )
                    # PSUM drain: DVE, except the tail i-block where the
                    # Activation engine is out of exp work and takes half;
                    # tail halves DMA out independently on two queues so the
                    # final transfer is short and issued early
                    if dma_eng is None:
                        # tail: ship each half as soon as it's cast, on its
                        # own queue, so the final drain overlaps compute
                        nc.vector.tensor_copy(
                            out=box['t'][:, mf * 512:(mf + 1) * 512], in_=ps)
                        (nc.sync if mf == 0 else nc.gpsimd).dma_start(
                            out_d[wt * 128:(wt + 1) * 128,
                                  mf * 512:(mf + 1) * 512],
                            box['t'][:, mf * 512:(mf + 1) * 512])
                    else:
                        nc.vector.tensor_copy(
                            out=box['t'][:, mf * 512:(mf + 1) * 512], in_=ps)
                        if mf == 1:
                            dma_eng.dma_start(
                                out_d[wt * 128:(wt + 1) * 128, :], box['t'])
                # per-c2 granularity: each (mf, c2) is one filler unit so
                # pops interleave at 213ns steps; c2=0 opens the chain
                def half(mf, c2):
                    if mf == 0 and c2 == 0:
                        box['t'] = work.tile([128, C], BF16, name="ost",
                                             tag="ost", bufs=3)
                    if c2 == 0:
                        tag = "nud" if (dma_eng is None and mf == 1) else "mix"
                        box[mf] = pp.tile([128, 512], F32, tag=tag,
                                          name="ps_mix", bufs=2)
                    nc.tensor.matmul(
                        box[mf],
                        lhsT=nudT[c2][:, wt * 128:(wt + 1) * 128],
                        rhs=wmT[:, c2, mf * 512:(mf + 1) * 512],
                        start=(c2 == 0), stop=(c2 == 1),
                    )
                    if c2 == 1:
                        nc.vector.tensor_copy(
                            out=box['t'][:, mf * 512:(mf + 1) * 512],
                            in_=box[mf])
                        if mf == 1:
                            (nc.sync if dma_eng is None else
                             dma_eng).dma_start(
                                out_d[wt * 128:(wt + 1) * 128, :], box['t'])
                if dma_eng is None:
                    return [(True, lambda mf=mf: mmpair(mf))
                            for mf in range(2)]
                return [(c2 == 0, lambda mf=mf, c2=c2: half(mf, c2))
                        for mf in range(2) for c2 in range(2)]

            def pop_filler():
                filler.pop(0)[1]()

            def flush_until(pred):
                while filler and not pred():
                    pop_filler()

            def flush_chain_boundary():
                # emit until the queue head would OPEN a new psum chain, so
                # no half-accumulated "mix"-tag tile is left holding a slot
                # that the rbc/mix section below would deadlock on.
                while filler and not filler[0][0]:
                    pop_filler()

            def pop_n_to_boundary(n):
                for _ in range(n):
                    if not filler:
                        break
                    pop_filler()
                flush_chain_boundary()

            for wt in range(4):
                for _, f in emit_proj(wt, tag="sc"):
                    f()
            for g in range(2):
                for _, f in emit_gt(g, 0, tag="sc"):
                    f()

            # ---- main loop: attention -> normalize -> mix, per i-block -----
            # ET[j, i] = exp(scores[j, i]/8) * (j <= i), accumulated into
            # nudged^T via att^T @ proj; the ones-column gives row sums.
            nudT = [big.tile([128, W], BF16, name=f"nudT{g}", tag=f"nudT{g}")
                    for g in range(2)]

            def emit_norm(i2, g, rbc_tag="mix"):
                # one K=33 selector matmul broadcasts both heads' recip
                # rows onto their 64-partition blocks, then one multiply
                # normalizes the whole 128-partition i-block
                i2s = slice(i2 * 512, (i2 + 1) * 512)
                if rbc_tag == "sc":
                    rbc = pp.tile([128, 2, 512], F32, tag="sc",
                                  name="rbc_ps", bufs=2)[:, 0]
                else:
                    rbc = pp.tile([128, 512], F32, tag=rbc_tag,
                                  name="rbc_ps", bufs=2)
                nc.tensor.matmul(
                    rbc,
                    lhsT=sel33[0:33],
                    rhs=rw_all[0:33, 2 * i2 + g],
                    start=True, stop=True,
                )
                nc.vector.tensor_tensor(
                    nudT[g][:, i2s],
                    nudT[g][:, i2s],
                    rbc,
                    mybir.AluOpType.mult,
                )

            def emit_norm_mix(i2, tail=False):
                # normalize + mix for i-block i2, deferred into the middle of
                # the next i-block's attention so the recip/drain chain on
                # DVE/Act overlaps a whole jp loop instead of stalling the PE
                flush_chain_boundary()
                if tail:
                    # head-pair 0 was normalized before the last jp loop, so
                    # the first three mix tiles' c2=0 accumulations are legal
                    # PE work while the final recips run on DVE.  6 opened
                    # banks (mix 2 + nud 2 + one 2-bank "sc" alloc) plus the
                    # final rbc on the other "sc" alloc fill all of PSUM.
                    opened = []
                    for wt in (4 * i2, 4 * i2 + 1, 4 * i2 + 2):
                        ost = work.tile([128, C], BF16, name="ost",
                                        tag="ost", bufs=3)
                        sc_ps = (pp.tile([128, 2, 512], F32, tag="sc",
                                         name="ps_mixo_sc", bufs=2)
                                 if wt == 4 * i2 + 2 else None)
                        for mf in range(2):
                            if sc_ps is not None:
                                ps = sc_ps[:, mf]
                            else:
                                ps = pp.tile([128, 512], F32,
                                             tag="mix" if mf == 0 else "nud",
                                             name="ps_mixo", bufs=2)
                            nc.tensor.matmul(
                                ps,
                                lhsT=nudT[0][:, wt * 128:(wt + 1) * 128],
                                rhs=wmT[:, 0, mf * 512:(mf + 1) * 512],
                                start=True, stop=False,
                            )
                            opened.append((wt, mf, ps, ost))
                    emit_norm(i2, 1, rbc_tag="sc")
                    dmaq = [nc.sync, nc.scalar, nc.gpsimd]
                    for qi, (wt, mf, ps, ost) in enumerate(opened):
                        nc.tensor.matmul(
                            ps,
                            lhsT=nudT[1][:, wt * 128:(wt + 1) * 128],
                            rhs=wmT[:, 1, mf * 512:(mf + 1) * 512],
                            start=False, stop=True,
                        )
                        eng = (nc.vector.tensor_copy if mf == 0 else
                               lambda out, in_: nc.scalar.activation(
                                   out, in_,
                                   mybir.ActivationFunctionType.Copy,
                                   scale=1.0))
                        eng(out=ost[:, mf * 512:(mf + 1) * 512], in_=ps)
                        dmaq[qi % 3].dma_start(
                            out_d[wt * 128:(wt + 1) * 128,
                                  mf * 512:(mf + 1) * 512],
                            ost[:, mf * 512:(mf + 1) * 512])
                    for _, f in emit_mix(4 * i2 + 3, None):
                        f()
                    return
                for g in range(2):
                    emit_norm(i2, g)
                # mix: half the row-tiles ride the filler queue, the rest are
                # emitted directly (the tail block drains everything now)
                for wt in range(4 * i2, 4 * i2 + 4):
                    filler.extend(emit_mix(wt, nc.sync))

            def emit_scores_for(iFx, gx, jp):
                # columns < 128*d are fully causal-masked for this
                # j-block: never computed, never accumulated.
                d = jp - 4 * iFx
                lo = 128 * d if d > 0 else 0
                sc = pp.tile([128, 2, 512], F32, tag="sc",
                             name="sc_ps", bufs=2)
                for s in range(2):
                    nc.tensor.matmul(
                        sc[:, s, lo:],
                        lhsT=GT[gx][64 * s:64 * s + 64,
                                    jp * 128:(jp + 1) * 128],
                        rhs=GT[gx][64 * s:64 * s + 64,
                                   iFx * 512 + lo:(iFx + 1) * 512],
                        start=True, stop=True,
                    )
                et = etp.tile([128, 2, 512], BF16, name="et", tag="et")
                nc.scalar.activation(et[:, :, lo:], sc[:, :, lo:],
                                     Exp, scale=SCALE)
                if d >= 0:
                    # zero the strictly-upper part of the 128-wide
                    # diagonal crossing strip (cols [lo, lo+128))
                    nc.gpsimd.affine_select(
                        out=et[:, :, lo:lo + 128],
                        in_=et[:, :, lo:lo + 128],
                        compare_op=ge, fill=0.0,
                        base=0, channel_multiplier=-1,
                        pattern=[[0, 2], [1, 128]],
                    )
                return et

            for iF in range(4):
                njp = 4 * iF + 4
                flush_until(lambda: (0, iF) in gt_done and (1, iF) in gt_done)
                if iF + 1 < 4:
                    for wt in range(4 * iF + 4, 4 * iF + 8):
                        for _, f in emit_proj(wt, tag="mix"):
                            f()
                    for g in range(2):
                        filler.extend(emit_gt(g, iF + 1, tag="mix"))
                for g in range(2):
                    npair = [
                        pp.tile([128, 512], F32, tag="nud", name=f"nud_ps{s}",
                                bufs=2)
                        for s in range(2)
                    ]

                    def emit_npair(jp, et):
                        d = jp - 4 * iF
                        lo = 128 * d if d > 0 else 0
                        for s in range(2):
                            nc.tensor.matmul(
                                npair[s][:K + 1, lo:],
                                lhsT=proj[jp][:, 2 * g + s],
                                rhs=et[:, s, lo:],
                                start=(jp == 0), stop=(jp == njp - 1),
                            )

                    # two-deep software pipeline: scores(jp+2) is emitted
                    # ahead of npair(jp) so the Exp and the diagonal mask
                    # never gate the PE; one filler matmul per jp rides out
                    # the Exp per-block overhead.
                    ets = [emit_scores_for(iF, g, 0),
                           emit_scores_for(iF, g, 1)]
                    if g == 0 and iF > 0:
                        # the previous i-block's npair PSUM frees only after
                        # its recip/drain chain; filler rides that out
                        pop_n_to_boundary(4)
                    for jp in range(njp):
                        if jp + 2 < njp:
                            ets.append(emit_scores_for(iF, g, jp + 2))
                        flush_until(lambda: proj[jp] is not None)
                        emit_npair(jp, ets[jp])
                        if filler and (iF < 3 or len(filler) > 6):
                            pop_filler()

                    # stage both rowsum rows (parallel DVE+Act copies) so
                    # the npair PSUM banks free in ~0.5us.  The reciprocal
                    # is 4 short DVE ops (bitwise-NOT seed + one tuned
                    # Newton step, max rel err ~2e-3 -- invisible under the
                    # bf16 data plane) instead of the microcoded RECIPROCAL
                    # whose ~3.4us runtime stalled whatever queued behind it.
                    pl = (2 * iF + g) % 2
                    nc.vector.tensor_copy(out=rsum2[0:1, pl],
                                          in_=npair[0][K:K + 1])
                    nc.scalar.activation(
                        rsum2[32:33, pl], npair[1][K:K + 1],
                        mybir.ActivationFunctionType.Copy, scale=1.0)

                    def emit_recip(iF=iF, g=g, pl=pl):
                        C0, C1 = -0.23549792, 2.0017324
                        x = rsum2[0:33, pl]
                        t0 = work.tile([33, 512], F32, name="rt0",
                                       tag="rt0", bufs=2)
                        w1 = work.tile([33, 512], F32, name="rw1",
                                       tag="rw1", bufs=2)
                        nc.vector.tensor_scalar(
                            out=t0.bitcast(I32), in0=x.bitcast(I32),
                            scalar1=0, scalar2=None,
                            op0=mybir.AluOpType.bitwise_not)
                        nc.vector.tensor_tensor(
                            w1, x, t0, mybir.AluOpType.mult)
                        nc.vector.tensor_scalar(
                            out=w1, in0=w1, scalar1=C0, scalar2=C1,
                            op0=mybir.AluOpType.mult,
                            op1=mybir.AluOpType.subtract)
                        with nc.allow_low_precision(
                                reason="f32r recip feeds f32r matmul"):
                            nc.vector.scalar_tensor_tensor(
                                out=rw_all[0:33, 2 * iF + g],
                                in0=w1, scalar=-C0, in1=t0,
                                op0=mybir.AluOpType.mult,
                                op1=mybir.AluOpType.mult)

                    for s in range(2):
                        # g=1 drains land in the i-block boundary window
                        # where Act has no exp backlog but DVE is piled up
                        dst = nudT[g][64 * s:64 * s + 64,
                                      iF * 512:(iF + 1) * 512]
                        if g == 1:
                            nc.scalar.activation(
                                dst, npair[s][:K],
                                mybir.ActivationFunctionType.Copy, scale=1.0)
                        else:
                            nc.vector.tensor_copy(out=dst, in_=npair[s][:K])

                    emit_recip()
                    if g == 0:
                        if iF > 0:
                            emit_norm_mix(iF - 1)
                        if iF == 3:
                            # normalize the last block's first head-pair
                            # before the second pair's jp loop, off the
                            # tail chain
                            flush_chain_boundary()
                            emit_norm(3, 0)


            flush_until(lambda: False)
            emit_norm_mix(3, tail=True)
    if split_waits:
        _split_waits(nc)
    return nc


_NC_CACHE = None


def _get_nc():
    global _NC_CACHE
    if _NC_CACHE is None:
        _NC_CACHE = build_nc()
    return _NC_CACHE


def make_in_maps(in_sequence_bwc, W_proj, pre_metric, W_mix):
    bf = ml_dtypes.bfloat16
    # weight-only preprocessing: P_h = softmax(tril(pre_metric_h)/sqrt(k));
    # G^T = P^T @ proj^T is built on-device from xbar-transposed proj.
    pmf = np.asarray(pre_metric, np.float64)
    pmf = np.where(np.tril(np.ones((K, K), bool)), pmf, -np.inf) / np.sqrt(K)
    pmf = pmf - pmf.max(-1, keepdims=True)
    P = np.exp(pmf)
    P /= P.sum(-1, keepdims=True)                       # [NH, K, K]
    WgT = np.einsum('nkc,nkl->nlc', W_proj.reshape(NH, K, C).astype(np.float64),
                    P)                                  # [NH, K(l), C]
    in_maps = []
    for core in range(8):
        b, hg = core // 4, core % 4
        cs = slice(CPC * hg, CPC * (hg + 1))
        pmat = np.zeros((128, 2, 128), np.float64)      # blockdiag P pairs
        for hl in range(4):
            g_, s_ = hl // 2, hl % 2
            pmat[64 * s_:64 * s_ + 64, g_,
                 64 * s_:64 * s_ + 64] = P[4 * hg + hl]
        wg = WgT[4 * hg:4 * hg + 4].reshape(CPC, C).T   # [C, CPC]
        # pack to the kernel's SBUF layouts so DMA lines are contiguous:
        #   xTp[h, q, p, g, w'] = x^T[128*(4h+g)+p, 256q+w']
        #   wpT/wg[p, g, m]     = (.)[128g+p, m]
        #   wmT[ci, co, m]      = W_mix[:, cs].T[128co+ci, m]
        xbT = in_sequence_bwc[b].T.astype(bf)           # [C, W]
        xTp = xbT.reshape(2, 4, 128, 8, 256).transpose(0, 3, 2, 1, 4)
        wpTp = W_proj[cs, :].T.reshape(8, 128, CPC).transpose(1, 0, 2)
        wgp = wg.reshape(8, 128, CPC).transpose(1, 0, 2)
        wmTp = W_mix[:, cs].T.reshape(2, 128, C).transpose(1, 0, 2)
        in_maps.append({
            "xTp": np.ascontiguousarray(xTp),
            "wpT": np.ascontiguousarray(wpTp.astype(bf)),
            "pmat": np.ascontiguousarray(pmat.astype(bf)),
            "wg": np.ascontiguousarray(wgp.astype(bf)),
            "wmT": np.ascontiguousarray(wmTp.astype(bf)),
        })
    return in_maps


def combine_results(results):
    out = np.zeros((B, W, C), np.float32)
    for core in range(8):
        out[core // 4] += np.asarray(results[core]["partial"], np.float32)
    return out


def kernel(in_sequence_bwc, W_proj, pre_metric, W_mix):
    nc = _get_nc()
    in_maps = make_in_maps(
        np.asarray(in_sequence_bwc), np.asarray(W_proj),
        np.asarray(pre_metric), np.asarray(W_mix),
    )
    res = run_bass_kernel_spmd(nc, in_maps, list(range(8))).results
    return combine_results(res)



# revision 43
# speedup vs baseline: 1.0268x; 1.0120x over previous
"""MetricSelfAttention TRN2 kernel, v3 (bf16 data plane).

Reference computation (b=2, w=2048, c=1024, n=16 heads, k=64):
    P        = softmax(tril_mask(pre_metric) / sqrt(k))      per head [k,k]
    metric   = P @ P^T
    proj     = x @ W_proj^T                                  (Q = K = V)
    scores   = (proj_h @ metric_h @ proj_h^T) / sqrt(k)      causal-masked softmax
    out      = (att @ proj) @ W_mix^T

scores = G @ G^T with G = proj_h @ P_h; transposed attention blocks
ET[j, i] = exp(s[j, i]/sqrt(k)) * causal(j <= i) are built directly in the
G^T layout, a ones-column appended to proj yields softmax row-sums from the
same matmul that accumulates att^T @ proj, and normalization is folded in
before the final mix matmul.

v2 changes vs v1:
  - bf16 everywhere off the PSUM accumulators: halves DMA traffic, removes
    the narrow-tile fp32r matmul penalty, and doubles several DVE ops.
  - G^T is computed straight from x^T with pre-multiplied weights
    Wg = W_proj_h^T @ P_h (tiny PE transform), eliminating the projT
    recompute pass and its PSUM->SBUF copies.
  - software-pipelined attention loop: scores(jp+1) is emitted before
    att^T@proj(jp), and filler matmuls (next i-block's proj/GT chains,
    previous i-block's mix) are drained one per jp so the PE rides out the
    per-block Exp overhead on the Activation engine.
  - causal mask applied by gpsimd affine_select (Pool engine), PSUM->SBUF
    copies split across DVE and Pool, reciprocal reads row-sums directly
    from PSUM, input DMA batched and spread across idle engine queues.

v3 changes vs v2 (193us -> 148us):
  - softmax reciprocal rebuilt: both heads' row-sum rows are staged off
    PSUM in ~0.5us (parallel DVE+Act copies, freeing the npair banks the
    PE was stalling on), then ONE 33-lane bitwise-NOT-seeded Newton
    reciprocal (4 short DVE ops, ~2e-3 max err) replaces two 3.4us
    single-lane microcoded RECIPROCALs whose latency serialized the rbc
    matmuls, GT casts and PSUM recycling behind them (~30us of PE idle).
  - host-side input repack: every dram tensor is laid out exactly as its
    SBUF tile ([p][g][m] etc.), so DMA lines are 1-4KB contiguous instead
    of 512B -- less descriptor pressure, faster startup fill.
  - tail output is shipped per-[128,512] half on three queues as soon as
    each cast lands, instead of whole [128,1024] tiles on one queue.

Sharding over 8 cores: core = (batch, head-group of 4).  Each core computes
a [2048, 1024] bf16 partial of the mix output for its 256 channels; the
host sums the 4 partials per batch in fp32.
"""

import numpy as np
import ml_dtypes

import concourse.bass as bass
import concourse.mybir as mybir
import concourse.tile as tile
from concourse.bass_utils import run_bass_kernel_spmd

B, W, C, NH, K = 2, 2048, 1024, 16, 64
HPC = 4            # heads per core
CPC = HPC * K      # 256 channels per core
F32 = mybir.dt.float32
F32R = mybir.dt.float32r
I32 = mybir.dt.int32
BF16 = mybir.dt.bfloat16
SCALE = 1.0 / 8.0  # 1/sqrt(K)


def _split_waits(nc, max_waits=1):
    """Hoist extra sem waits onto wait-only EventSemaphore carriers.

    The walrus build here rejects any instruction carrying more than one
    sync wait ("Too many sync wait commands"), while Tile's add_semaphores
    freely packs several waits onto one instruction.  An EVSEM executes on
    the engine's sequencer, so program order still gates the instruction
    that originally carried the waits.
    """
    n_new = 0
    for f in nc.m.functions:
        for b in f.blocks:
            out = []
            changed = False
            for inst in b.instructions:
                si = inst.sync_info
                if si is not None and si.on_wait and len(si.on_wait) > max_waits:
                    waits = list(si.on_wait)
                    for w in waits[:-max_waits]:
                        n_new += 1
                        ev = mybir.InstEventSemaphore(
                            name=f"splitw_{n_new}_{inst.name}",
                            engine=inst.engine,
                            ins=[], outs=[],
                            sync_info=mybir.SyncInfo(on_wait=[w], on_update=[]),
                        )
                        out.append(ev)
                        changed = True
                    si.on_wait = waits[-max_waits:]
                out.append(inst)
            if changed:
                b.instructions = out
    return n_new


def build_nc(split_waits=True):
    # input layouts are pre-packed on the host to match the SBUF tiles
    # exactly, so every DMA line is >=1KB contiguous (512B lines ran the
    # fabric at ~half rate and tripled the descriptor count)
    nc = bass.Bass()
    xT_d = nc.dram_tensor("xTp", [2, 8, 128, 4, 256], BF16,
                          kind="ExternalInput")
    wpT_d = nc.dram_tensor("wpT", [128, 8, CPC], BF16, kind="ExternalInput")
    pmat_d = nc.dram_tensor("pmat", [128, 2, 128], BF16, kind="ExternalInput")
    wg_d = nc.dram_tensor("wg", [128, 8, CPC], BF16, kind="ExternalInput")
    wmT_d = nc.dram_tensor("wmT", [128, 2, C], BF16, kind="ExternalInput")
    out_d = nc.dram_tensor("partial", [W, C], BF16, kind="ExternalOutput")

    ge = mybir.AluOpType.is_ge
    Exp = mybir.ActivationFunctionType.Exp

    with tile.TileContext(nc) as tc:
        with (
            tc.tile_pool(name="big", bufs=1) as big,
            tc.tile_pool(name="work", bufs=2) as work,
            tc.tile_pool(name="et", bufs=6) as etp,
            tc.tile_pool(name="pp", bufs=1, space="PSUM") as pp,
        ):
            # ---- input loads: arrival priority is wpT + Wg + the first
            # 512 cols of x^T (proj and GT wf0), then the rest of x^T in
            # 256-col pieces, W_mix^T last (first needed ~25us in).  Issuance
            # is spread over the SP and Activation queues; everything x^T
            # moves in 256-col pieces, which the DMA fabric services at
            # twice the byte-rate of 512-col slices.
            wpT = big.tile([128, 8, CPC], BF16, name="wpT")
            nc.sync.dma_start(wpT[:, 0:2], wpT_d[:, 0:2])
            xTh = [big.tile([128, 4, W], BF16, name=f"xT{h}", tag=f"xT{h}")
                   for h in range(2)]
            xT = [xTh[ct // 4][:, ct % 4] for ct in range(8)]
            P_t = big.tile([128, 2, 128], BF16, name="P_t")
            Wg = big.tile([128, 8, CPC], BF16, name="Wg")
            wmT = big.tile([128, 2, C], BF16, name="wmT")
            nc.scalar.dma_start(xTh[0][:, :, 0:256], xT_d[0, 0])
            nc.sync.dma_start(xTh[1][:, :, 0:256], xT_d[1, 0])
            nc.gpsimd.dma_start(wpT[:, 2:8], wpT_d[:, 2:8])
            nc.gpsimd.dma_start(Wg, wg_d[:, :, :])
            nc.scalar.dma_start(xTh[0][:, :, 256:512], xT_d[0, 1])
            nc.sync.dma_start(xTh[1][:, :, 256:512], xT_d[1, 1])
            for q in range(2, 8):
                nc.scalar.dma_start(xTh[0][:, :, 256 * q:256 * (q + 1)],
                                    xT_d[0, q])
                nc.sync.dma_start(xTh[1][:, :, 256 * q:256 * (q + 1)],
                                  xT_d[1, q])
            nc.sync.dma_start(P_t, pmat_d[:, :, :])
            nc.gpsimd.dma_start(wmT, wmT_d[:, :, :])

            # a throwaway Exp warms the activation table while DMAs stream
            warm = big.tile([1, 2], F32, name="warm")
            nc.vector.memset(warm, 1.0)
            nc.scalar.activation(warm, warm, Exp, scale=1.0)
            # recip-broadcast setup: recip rows live on partitions 0 and 32
            # of a plane per (i-block, head-pair), and one K=33 selector
            # matmul broadcasts both onto their 64-partition blocks.  Both
            # heads' rowsums are staged into one [33,512] tile (partitions
            # 0/32; lanes 1..31 hold 1.0 from a one-time fill) so a single
            # 33-lane reciprocal covers both heads and the rbc matmul's dead
            # lanes see finite recip(1.0), not NaN-tainted garbage.
            sel33 = big.tile([64, 128], F32R, name="sel33")
            nc.vector.memset(sel33.bitcast(F32), 0.0)
            nc.vector.memset(sel33.bitcast(F32)[0:1, 0:K], 1.0)
            nc.vector.memset(sel33.bitcast(F32)[32:33, K:128], 1.0)
            rw_all = big.tile([64, 8, 512], F32R, name="rw_all")
            rsum2 = big.tile([33, 2, 512], F32, name="rsum2")
            nc.gpsimd.memset(rsum2[0:32, :], 1.0)

            # ---- proj row-tiles and GT = (x @ Wg)^T -------------------------
            # proj tile wt: [128 w-rows, (h, k)] + ones column for row-sums.
            # Next i-block's proj/GT chains and the previous i-block's mix
            # tiles flow through a FIFO filler queue drained one matmul per
            # jp of the attention loop (PSUM via the then-idle "mix" tag).
            proj = [None] * 16
            tr_by_wt = {}
            projT_all = big.tile([128, 2, W], BF16, name="projT")
            GT = [big.tile([128, W], BF16, name=f"GT{g}", tag=f"GT{g}")
                  for g in range(2)]
            gt_done = set()
            filler = []          # items: (opens_chain, fn)

            def emit_proj(wt, tag):
                box = {}

                def mm(ct):
                    if ct == 0:
                        ps = pp.tile([128, 2, 512], F32, tag=tag,
                                     name="ps_proj", bufs=2) \
                            if tag == "sc" else \
                            pp.tile([128, 512], F32, tag=tag,
                                    name="ps_projf", bufs=2)
                        box['ps'] = ps[:, 0] if tag == "sc" else ps
                    nc.tensor.matmul(
                        box['ps'][:, 0:CPC],
                        lhsT=xT[ct][:, wt * 128:(wt + 1) * 128],
                        rhs=wpT[:, ct],
                        start=(ct == 0), stop=(ct == 7),
                    )
                    if ct == 7:
                        pt = big.tile([128, HPC, K + 1], BF16,
                                      name=f"proj{wt}", tag=f"proj{wt}")
                        nc.vector.tensor_copy(
                            out=pt[:, :, 0:K],
                            in_=box['ps'][:, 0:CPC].rearrange(
                                "p (h k) -> p h k", k=K),
                        )
                        nc.vector.memset(pt[:, :, K:K + 1], 1.0)
                        proj[wt] = pt
                        if wt < 4:
                            return  # wf0's GT comes from the direct path
                        # contiguous value copy feeds the xbar transpose
                        # that builds projT[k', h, w] for the GT matmuls.
                        # InstDmaTransposeAnt is invisible to the tile dep
                        # tracker, so its edges are added explicitly.
                        pv = work.tile([128, CPC], BF16, name="pv",
                                       tag="pv", bufs=3)
                        pvc = nc.vector.tensor_copy(out=pv,
                                                    in_=box['ps'][:, 0:CPC])
                        tr = nc.sync.dma_start_transpose(
                            projT_all[:, :, wt * 128:(wt + 1) * 128],
                            pv[:, :])
                        bass._add_dep_helper(
                            tr.ins, pvc.ins, sync=True,
                            reason="xbar transpose reads pv")
                        if wt - 3 in tr_by_wt:
                            bass._add_dep_helper(
                                pvc.ins, tr_by_wt[wt - 3].ins, sync=True,
                                reason="pv slot recycle")
                        tr_by_wt[wt] = tr
                return [(ct == 0, lambda ct=ct: mm(ct)) for ct in range(8)]

            def emit_gt(g, wf, tag):
                # wf=0 (startup): direct Wg^T @ x^T contraction -- projT
                # isn't ready until the input fabric drains.  wf>=1: two
                # tiny K=64 P^T @ projT matmuls per head-pair instead of
                # the full x^T re-contraction.
                if wf == 0:
                    box = {}

                    def mm0(ct):
                        if ct == 0:
                            ps = pp.tile([128, 2, 512], F32, tag=tag,
                                         name="ps_gt", bufs=2)
                            box['ps'] = ps[:, 0]
                        nc.tensor.matmul(
                            box['ps'],
                            lhsT=Wg[:, ct, 128 * g:128 * (g + 1)],
                            rhs=xT[ct][:, 0:512],
                            start=(ct == 0), stop=(ct == 7),
                        )
                        if ct == 7:
                            nc.vector.tensor_copy(
                                out=GT[g][:, 0:512], in_=box['ps'])
                            gt_done.add((g, 0))
                    return [(ct == 0, lambda ct=ct: mm0(ct))
                            for ct in range(8)]

                def unit():
                    if tag == "sc":
                        ps = pp.tile([128, 2, 512], F32, tag="sc",
                                     name="ps_gt", bufs=2)[:, 0]
                    else:
                        ps = pp.tile([128, 512], F32, tag=tag,
                                     name="ps_gtf", bufs=2)
                    mm = nc.tensor.matmul(
                        ps,
                        lhsT=P_t[:, g],
                        rhs=projT_all[:, g, wf * 512:(wf + 1) * 512],
                        start=True, stop=True,
                    )
                    bass._add_dep_helper(
                        mm.ins, tr_by_wt[4 * wf + 3].ins, sync=True,
                        reason="projT chunk ready (same-queue FIFO)")
                    nc.vector.tensor_copy(
                        out=GT[g][:, wf * 512:(wf + 1) * 512], in_=ps)
                    gt_done.add((g, wf))
                return [(True, unit)]

            def emit_mix(wt, dma_eng):
                box = {}

                def mmpair(mf):
                    if mf == 0:
                        box['t'] = work.tile([128, C], BF16, name="ost",
                                             tag="ost", bufs=3)
                    # the tail block also rotates through the then-idle
                    # "nud" banks so drains never block the matmul pairs
                    tag = "nud" if (dma_eng is None and mf == 1) else "mix"
                    ps = pp.tile([128, 512], F32, tag=tag, name="ps_mix",
                                 bufs=2)
                    for c2 in range(2):
                        nc.tensor.matmul(
                            ps,
                            lhsT=nudT[c2][:, wt * 128:(wt + 1) * 128],
                            rhs=wmT[:, c2, mf * 512:(mf + 1) * 512],
                            start=(c2 == 0), stop=(c2 == 1),
                        )
                    # PSUM drain: DVE, except the tail i-block where the
                    # Activation engine is out of exp work and takes half;
                    # tail halves DMA out independently on two queues so the
                    # final transfer is short and issued early
                    if dma_eng is None:
                        # tail: ship each half as soon as it's cast, on its
                        # own queue, so the final drain overlaps compute
                        nc.vector.tensor_copy(
                            out=box['t'][:, mf * 512:(mf + 1) * 512], in_=ps)
                        (nc.sync if mf == 0 else nc.gpsimd).dma_start(
                            out_d[wt * 128:(wt + 1) * 128,
                                  mf * 512:(mf + 1) * 512],
                            box['t'][:, mf * 512:(mf + 1) * 512])
                    else:
                        nc.vector.tensor_copy(
                            out=box['t'][:, mf * 512:(mf + 1) * 512], in_=ps)
                        if mf == 1:
                            dma_eng.dma_start(
                                out_d[wt * 128:(wt + 1) * 128, :], box['t'])
                # per-c2 granularity: each (mf, c2) is one filler unit so
                # pops interleave at 213ns steps; c2=0 opens the chain
                def half(mf, c2):
                    if mf == 0 and c2 == 0:
                        box['t'] = work.tile([128, C], BF16, name="ost",
                                             tag="ost", bufs=3)
                    if c2 == 0:
                        tag = "nud" if (dma_eng is None and mf == 1) else "mix"
                        box[mf] = pp.tile([128, 512], F32, tag=tag,
                                          name="ps_mix", bufs=2)
                    nc.tensor.matmul(
                        box[mf],
                        lhsT=nudT[c2][:, wt * 128:(wt + 1) * 128],
                        rhs=wmT[:, c2, mf * 512:(mf + 1) * 512],
                        start=(c2 == 0), stop=(c2 == 1),
                    )
                    if c2 == 1:
                        nc.vector.tensor_copy(
                            out=box['t'][:, mf * 512:(mf + 1) * 512],
                            in_=box[mf])
                        if mf == 1:
                            (nc.sync if dma_eng is None else
                             dma_eng).dma_start(
                                out_d[wt * 128:(wt + 1) * 128, :], box['t'])
                if dma_eng is None:
                    return [(True, lambda mf=mf: mmpair(mf))
                            for mf in range(2)]
                return [(c2 == 0, lambda mf=mf, c2=c2: half(mf, c2))
                        for mf in range(2) for c2 in range(2)]

            def pop_filler():
                filler.pop(0)[1]()

            def flush_until(pred):
                while filler and not pred():
                    pop_filler()

            def flush_chain_boundary():
                # emit until the queue head would OPEN a new psum chain, so
                # no half-accumulated "mix"-tag tile is left holding a slot
                # that the rbc/mix section below would deadlock on.
                while filler and not filler[0][0]:
                    pop_filler()

            def pop_n_to_boundary(n):
                for _ in range(n):
                    if not filler:
                        break
                    pop_filler()
                flush_chain_boundary()

            for wt in range(4):
                for _, f in emit_proj(wt, tag="sc"):
                    f()
            for g in range(2):
                for _, f in emit_gt(g, 0, tag="sc"):
                    f()

            # ---- main loop: attention -> normalize -> mix, per i-block -----
            # ET[j, i] = exp(scores[j, i]/8) * (j <= i), accumulated into
            # nudged^T via att^T @ proj; the ones-column gives row sums.
            nudT = [big.tile([128, W], BF16, name=f"nudT{g}", tag=f"nudT{g}")
                    for g in range(2)]

            def emit_norm(i2, g, rbc_tag="mix"):
                # one K=33 selector matmul broadcasts both heads' recip
                # rows onto their 64-partition blocks, then one multiply
                # normalizes the whole 128-partition i-block
                i2s = slice(i2 * 512, (i2 + 1) * 512)
                if rbc_tag == "sc":
                    rbc = pp.tile([128, 2, 512], F32, tag="sc",
                                  name="rbc_ps", bufs=2)[:, 0]
                else:
                    rbc = pp.tile([128, 512], F32, tag=rbc_tag,
                                  name="rbc_ps", bufs=2)
                nc.tensor.matmul(
                    rbc,
                    lhsT=sel33[0:33],
                    rhs=rw_all[0:33, 2 * i2 + g],
                    start=True, stop=True,
                )
                nc.vector.tensor_tensor(
                    nudT[g][:, i2s],
                    nudT[g][:, i2s],
                    rbc,
                    mybir.AluOpType.mult,
                )

            def emit_norm_mix(i2, tail=False):
                # normalize + mix for i-block i2, deferred into the middle of
                # the next i-block's attention so the recip/drain chain on
                # DVE/Act overlaps a whole jp loop instead of stalling the PE
                flush_chain_boundary()
                if tail:
                    # head-pair 0 was normalized before the last jp loop, so
                    # the first three mix tiles' c2=0 accumulations are legal
                    # PE work while the final recips run on DVE.  6 opened
                    # banks (mix 2 + nud 2 + one 2-bank "sc" alloc) plus the
                    # final rbc on the other "sc" alloc fill all of PSUM.
                    opened = []
                    for wt in (4 * i2, 4 * i2 + 1, 4 * i2 + 2):
                        ost = work.tile([128, C], BF16, name="ost",
                                        tag="ost", bufs=3)
                        sc_ps = (pp.tile([128, 2, 512], F32, tag="sc",
                                         name="ps_mixo_sc", bufs=2)
                                 if wt == 4 * i2 + 2 else None)
                        for mf in range(2):
                            if sc_ps is not None:
                                ps = sc_ps[:, mf]
                            else:
                                ps = pp.tile([128, 512], F32,
                                             tag="mix" if mf == 0 else "nud",
                                             name="ps_mixo", bufs=2)
                            nc.tensor.matmul(
                                ps,
                                lhsT=nudT[0][:, wt * 128:(wt + 1) * 128],
                                rhs=wmT[:, 0, mf * 512:(mf + 1) * 512],
                                start=True, stop=False,
                            )
                            opened.append((wt, mf, ps, ost))
                    emit_norm(i2, 1, rbc_tag="sc")
                    dmaq = [nc.sync, nc.scalar, nc.gpsimd]
                    for qi, (wt, mf, ps, ost) in enumerate(opened):
                        nc.tensor.matmul(
                            ps,
                            lhsT=nudT[1][:, wt * 128:(wt + 1) * 128],
                            rhs=wmT[:, 1, mf * 512:(mf + 1) * 512],
                            start=False, stop=True,
                        )
                        eng = (nc.vector.tensor_copy if mf == 0 else
                               lambda out, in_: nc.scalar.activation(
                                   out, in_,
                                   mybir.ActivationFunctionType.Copy,
                                   scale=1.0))
                        eng(out=ost[:, mf * 512:(mf + 1) * 512], in_=ps)
                        dmaq[qi % 3].dma_start(
                            out_d[wt * 128:(wt + 1) * 128,
                                  mf * 512:(mf + 1) * 512],
                            ost[:, mf * 512:(mf + 1) * 512])
                    for _, f in emit_mix(4 * i2 + 3, None):
                        f()
                    return
                for g in range(2):
                    emit_norm(i2, g)
                # mix: half the row-tiles ride the filler queue, the rest are
                # emitted directly (the tail block drains everything now)
                for wt in range(4 * i2, 4 * i2 + 4):
                    filler.extend(emit_mix(wt, nc.sync))

            def emit_scores_for(iFx, gx, jp):
                # columns < 128*d are fully causal-masked for this
                # j-block: never computed, never accumulated.
                d = jp - 4 * iFx
                lo = 128 * d if d > 0 else 0
                sc = pp.tile([128, 2, 512], F32, tag="sc",
                             name="sc_ps", bufs=2)
                for s in range(2):
                    nc.tensor.matmul(
                        sc[:, s, lo:],
                        lhsT=GT[gx][64 * s:64 * s + 64,
                                    jp * 128:(jp + 1) * 128],
                        rhs=GT[gx][64 * s:64 * s + 64,
                                   iFx * 512 + lo:(iFx + 1) * 512],
                        start=True, stop=True,
                    )
                et = etp.tile([128, 2, 512], BF16, name="et", tag="et")
                nc.scalar.activation(et[:, :, lo:], sc[:, :, lo:],
                                     Exp, scale=SCALE)
                if d >= 0:
                    # zero the strictly-upper part of the 128-wide
                    # diagonal crossing strip (cols [lo, lo+128))
                    nc.gpsimd.affine_select(
                        out=et[:, :, lo:lo + 128],
                        in_=et[:, :, lo:lo + 128],
                        compare_op=ge, fill=0.0,
                        base=0, channel_multiplier=-1,
                        pattern=[[0, 2], [1, 128]],
                    )
                return et

            pre_ets = None
            for iF in range(4):
                njp = 4 * iF + 4
                flush_until(lambda: (0, iF) in gt_done and (1, iF) in gt_done)
                if iF + 1 < 4:
                    for wt in range(4 * iF + 4, 4 * iF + 8):
                        for _, f in emit_proj(wt, tag="mix"):
                            f()
                    for g in range(2):
                        filler.extend(emit_gt(g, iF + 1, tag="mix"))
                for g in range(2):
                    npair = [
                        pp.tile([128, 512], F32, tag="nud", name=f"nud_ps{s}",
                                bufs=2)
                        for s in range(2)
                    ]

                    def emit_npair(jp, et):
                        d = jp - 4 * iF
                        lo = 128 * d if d > 0 else 0
                        for s in range(2):
                            nc.tensor.matmul(
                                npair[s][:K + 1, lo:],
                                lhsT=proj[jp][:, 2 * g + s],
                                rhs=et[:, s, lo:],
                                start=(jp == 0), stop=(jp == njp - 1),
                            )

                    # two-deep software pipeline: scores(jp+2) is emitted
                    # ahead of npair(jp) so the Exp and the diagonal mask
                    # never gate the PE; one filler matmul per jp rides out
                    # the Exp per-block overhead.
                    if pre_ets is not None:
                        ets = pre_ets
                        pre_ets = None
                    else:
                        ets = [emit_scores_for(iF, g, 0),
                               emit_scores_for(iF, g, 1)]
                    if g == 0 and iF > 0:
                        # the previous i-block's npair PSUM frees only after
                        # its recip/drain chain; filler rides that out
                        pop_n_to_boundary(4)
                    for jp in range(njp):
                        if jp + 2 < njp:
                            ets.append(emit_scores_for(iF, g, jp + 2))
                        flush_until(lambda: proj[jp] is not None)
                        emit_npair(jp, ets[jp])
                        if filler and (iF < 3 or len(filler) > 6):
                            pop_filler()

                    # stage both rowsum rows (parallel DVE+Act copies) so
                    # the npair PSUM banks free in ~0.5us.  The reciprocal
                    # is 4 short DVE ops (bitwise-NOT seed + one tuned
                    # Newton step, max rel err ~2e-3 -- invisible under the
                    # bf16 data plane) instead of the microcoded RECIPROCAL
                    # whose ~3.4us runtime stalled whatever queued behind it.
                    pl = (2 * iF + g) % 2
                    nc.vector.tensor_copy(out=rsum2[0:1, pl],
                                          in_=npair[0][K:K + 1])
                    nc.scalar.activation(
                        rsum2[32:33, pl], npair[1][K:K + 1],
                        mybir.ActivationFunctionType.Copy, scale=1.0)

                    def emit_recip(iF=iF, g=g, pl=pl):
                        C0, C1 = -0.23549792, 2.0017324
                        x = rsum2[0:33, pl]
                        t0 = work.tile([33, 512], F32, name="rt0",
                                       tag="rt0", bufs=2)
                        w1 = work.tile([33, 512], F32, name="rw1",
                                       tag="rw1", bufs=2)
                        nc.vector.tensor_scalar(
                            out=t0.bitcast(I32), in0=x.bitcast(I32),
                            scalar1=0, scalar2=None,
                            op0=mybir.AluOpType.bitwise_not)
                        nc.vector.tensor_tensor(
                            w1, x, t0, mybir.AluOpType.mult)
                        nc.vector.tensor_scalar(
                            out=w1, in0=w1, scalar1=C0, scalar2=C1,
                            op0=mybir.AluOpType.mult,
                            op1=mybir.AluOpType.subtract)
                        with nc.allow_low_precision(
                                reason="f32r recip feeds f32r matmul"):
                            nc.vector.scalar_tensor_tensor(
                                out=rw_all[0:33, 2 * iF + g],
                                in0=w1, scalar=-C0, in1=t0,
                                op0=mybir.AluOpType.mult,
                                op1=mybir.AluOpType.mult)

                    for s in range(2):
                        # g=1 drains land in the i-block boundary window
                        # where Act has no exp backlog but DVE is piled up
                        dst = nudT[g][64 * s:64 * s + 64,
                                      iF * 512:(iF + 1) * 512]
                        if g == 1:
                            nc.scalar.activation(
                                dst, npair[s][:K],
                                mybir.ActivationFunctionType.Copy, scale=1.0)
                        else:
                            nc.vector.tensor_copy(out=dst, in_=npair[s][:K])

                    emit_recip()
                    if g == 0:
                        # pre-emit the second head-pair's first two score
                        # blocks: they depend only on this i-block's GT and
                        # fill the recip/norm-mix window with PE work, so
                        # Act's exp pipeline stays fed across the boundary
                        pre_ets = [emit_scores_for(iF, 1, 0),
                                   emit_scores_for(iF, 1, 1)]
                        if iF > 0:
                            emit_norm_mix(iF - 1)
                        if iF == 3:
                            # normalize the last block's first head-pair
                            # before the second pair's jp loop, off the
                            # tail chain
                            flush_chain_boundary()
                            emit_norm(3, 0)


            flush_until(lambda: False)
            emit_norm_mix(3, tail=True)
    if split_waits:
        _split_waits(nc)
    return nc


_NC_CACHE = None


def _get_nc():
    global _NC_CACHE
    if _NC_CACHE is None:
        _NC_CACHE = build_nc()
    return _NC_CACHE


def make_in_maps(in_sequence_bwc, W_proj, pre_metric, W_mix):
    bf = ml_dtypes.bfloat16
    # weight-only preprocessing: P_h = softmax(tril(pre_metric_h)/sqrt(k));
    # G^T = P^T @ proj^T is built on-device from xbar-transposed proj.
    pmf = np.asarray(pre_metric, np.float64)
    pmf = np.where(np.tril(np.ones((K, K), bool)), pmf, -np.inf) / np.sqrt(K)
    pmf = pmf - pmf.max(-1, keepdims=True)
    P = np.exp(pmf)
    P /= P.sum(-1, keepdims=True)                       # [NH, K, K]
    WgT = np.einsum('nkc,nkl->nlc', W_proj.reshape(NH, K, C).astype(np.float64),
                    P)                                  # [NH, K(l), C]
    in_maps = []
    for core in range(8):
        b, hg = core // 4, core % 4
        cs = slice(CPC * hg, CPC * (hg + 1))
        pmat = np.zeros((128, 2, 128), np.float64)      # blockdiag P pairs
        for hl in range(4):
            g_, s_ = hl // 2, hl % 2
            pmat[64 * s_:64 * s_ + 64, g_,
                 64 * s_:64 * s_ + 64] = P[4 * hg + hl]
        wg = WgT[4 * hg:4 * hg + 4].reshape(CPC, C).T   # [C, CPC]
        # pack to the kernel's SBUF layouts so DMA lines are contiguous:
        #   xTp[h, q, p, g, w'] = x^T[128*(4h+g)+p, 256q+w']
        #   wpT/wg[p, g, m]     = (.)[128g+p, m]
        #   wmT[ci, co, m]      = W_mix[:, cs].T[128co+ci, m]
        xbT = in_sequence_bwc[b].T.astype(bf)           # [C, W]
        xTp = xbT.reshape(2, 4, 128, 8, 256).transpose(0, 3, 2, 1, 4)
        wpTp = W_proj[cs, :].T.reshape(8, 128, CPC).transpose(1, 0, 2)
        wgp = wg.reshape(8, 128, CPC).transpose(1, 0, 2)
        wmTp = W_mix[:, cs].T.reshape(2, 128, C).transpose(1, 0, 2)
        in_maps.append({
            "xTp": np.ascontiguousarray(xTp),
            "wpT": np.ascontiguousarray(wpTp.astype(bf)),
            "pmat": np.ascontiguousarray(pmat.astype(bf)),
            "wg": np.ascontiguousarray(wgp.astype(bf)),
            "wmT": np.ascontiguousarray(wmTp.astype(bf)),
        })
    return in_maps


def combine_results(results):
    out = np.zeros((B, W, C), np.float32)
    for core in range(8):
        out[core // 4] += np.asarray(results[core]["partial"], np.float32)
    return out


def kernel(in_sequence_bwc, W_proj, pre_metric, W_mix):
    nc = _get_nc()
    in_maps = make_in_maps(
        np.asarray(in_sequence_bwc), np.asarray(W_proj),
        np.asarray(pre_metric), np.asarray(W_mix),
    )
    res = run_bass_kernel_spmd(nc, in_maps, list(range(8))).results
    return combine_results(res)

